# revision 15
# baseline (speedup 1.0000x reference)
"""Trainium2 Bass kernel for nn_AllAttLayer (cross-batch attention gating layer).

Reference computation (B=8, C=512, H=W=32, HW=1024):
    xf = x as [B, HW, C]
    q = xf @ Wq.T + bq ; k = xf @ Wk.T + bk
    scores = q.flat @ k.flat.T                  # [B*HW, B*HW]
    xw = max over each image's keys, mean over images   # [B*HW]
    xw = softmax(xw * C**-0.5 per image)        # [B, HW]
    out = (x * xw) @ W6.T + b6  (1x1 conv)      # == W6 @ (x * xw)

Sharding: core b owns image b (its 1024 queries). No collectives: the
host replicates the full x (fp8, DoubleRow layout, ROLLED per core so
the core's own image is always slot 0) and scaled fp8 WqT/WkT to every
core; each core computes every image's keys locally with fp8 DoubleRow
projections.

Everything is c-major ([C, HW]: channel on partitions, pixel on free
dim) so PE matmuls need no transposes.

Precision: q/k projections and score matmuls all run fp8e4 DoubleRow
(2 fp8 weights per PE cell -> effective K=256 per matmul, 2x bf16
throughput). Weights are host-scaled by 16 (subnormal avoidance) and
unscaled in the PSUM evacuation. The final conv runs bf16; reductions,
softmax and the output stay fp32.

Engine balance vs the previous revision: score PSUM tiles are consumed
PAIRWISE by a single DVE tensor_tensor_reduce (op0=max over the two
512-key halves, op1=max along the free dim, accum_out -> per-image max
column). This halves the DVE instruction count on the critical score
path (the old per-tile tensor_reduce made DVE the pacing engine at
~660ns/tile vs the PE's ~430ns/tile) and eliminates the separate
half-merge pass. Key-tile quantization stays on ScalarE. The gating is
applied to x (bf16 2x-mode DVE muls) BEFORE the final conv, which runs
at the tail when all PSUM banks are free.
"""

import sys
import numpy as np

for _p in ("/opt/trn_rl_repo",):
    if _p not in sys.path:
        sys.path.insert(0, _p)

B, C, H, W = 8, 512, 32, 32
HW = H * W              # 1024 pixels per image
NCORES = 8
CB = C // 128           # 4 channel blocks
G = 2                   # DoubleRow groups (K=256 each)
QB = HW // 128          # 8 query blocks per core
KH = 2                  # key halves (512 keys each)
NIMG = NCORES
SCALE = 1.0 / float(np.sqrt(C))

W_SCALE = 16.0          # host scales WqT/WkT by this before fp8
NDVE = 6                # query blocks per image consumed by DVE (rest: LSE)
LSE_T = 2.0             # temperature for the scalar-engine LSE path


def build_kernel():
    from concourse import bacc, tile, mybir

    f32 = mybir.dt.float32
    bf16 = mybir.dt.bfloat16
    fp8 = mybir.dt.float8e4
    DR = mybir.MatmulPerfMode.DoubleRow

    nc = bacc.Bacc("TRN2", target_bir_lowering=False, debug=False,
                   num_devices=NCORES)

    # per-core own image (bf16, for the final conv) and weights
    x_in = nc.dram_tensor("x", [C, HW], bf16, kind="ExternalInput").ap()
    w6t_in = nc.dram_tensor("w6t", [C, C], bf16, kind="ExternalInput").ap()
    # replicated full x (rolled: slot 0 = own image) and scaled WqT/WkT in
    # fp8 DoubleRow layouts
    x8_in = [nc.dram_tensor(f"x8g{g}", [128, 2 * NCORES * HW], fp8,
                            kind="ExternalInput").ap() for g in range(G)]
    wq8_in = [nc.dram_tensor(f"wq8g{g}", [128, 2 * C], fp8,
                             kind="ExternalInput").ap() for g in range(G)]
    wk8_in = [nc.dram_tensor(f"wk8g{g}", [128, 2 * C], fp8,
                             kind="ExternalInput").ap() for g in range(G)]
    bq_in = nc.dram_tensor("bq", [C, 1], f32, kind="ExternalInput").ap()
    bk_in = nc.dram_tensor("bk", [C, 1], f32, kind="ExternalInput").ap()
    b6_in = nc.dram_tensor("b6", [C, 1], f32, kind="ExternalInput").ap()
    out_ext = nc.dram_tensor("out", [C, HW], f32, kind="ExternalOutput").ap()
    dbg_ext = nc.dram_tensor("dbg", [128, 128], f32,
                             kind="ExternalOutput").ap()

    AF = mybir.ActivationFunctionType
    ALU = mybir.AluOpType
    AX = mybir.AxisListType

    def dr3(ap, span):
        """[128, G*span] tile AP -> [128, 2, span] DoubleRow view."""
        return ap.rearrange("p (i n) -> p i n", i=2, n=span)

    with tile.TileContext(nc) as tc:
        with tc.tile_pool(name="consts", bufs=1) as consts, \
             tc.tile_pool(name="wpool", bufs=1) as wpool, \
             tc.tile_pool(name="xpool", bufs=1) as xpool, \
             tc.tile_pool(name="qpool", bufs=1) as qpool, \
             tc.tile_pool(name="klpool", bufs=2) as klpool, \
             tc.tile_pool(name="redpool", bufs=1) as redpool, \
             tc.tile_pool(name="scrpool", bufs=4) as scrpool, \
             tc.tile_pool(name="outpool", bufs=2) as outpool, \
             tc.tile_pool(name="dram", bufs=1, space="DRAM") as dram, \
             tc.tile_pool(name="ps_s", bufs=6, space="PSUM") as ps_s, \
             tc.tile_pool(name="ps_m", bufs=2, space="PSUM") as ps_m:

            bias_sb = {}

            def load_bias(nm, src, eng):
                t = consts.tile([128, CB], f32, tag=f"{nm}_sb", name=f"{nm}_sb")
                for co in range(CB):
                    eng.dma_start(out=t[:, co:co + 1],
                                  in_=src[co * 128:(co + 1) * 128, :])
                bias_sb[nm] = t

            # ---- head loads ----
            # The scalar queue gets NO head DMAs (its engine time is needed
            # for evacuations from ~3us on). Small q-path inputs first so
            # the first matmul isn't starved, then x8 in per-image-pair
            # chunks so image 0's key projection begins while later images
            # stream in, then the tail-only inputs (x bf16, w6t, b6).
            wq8_sb, wk8_sb, x8_sb = [], [], []
            for g in range(G):
                t = wpool.tile([128, 2 * C], fp8, tag=f"wq8{g}", name=f"wq8{g}")
                nc.sync.dma_start(out=t[:], in_=wq8_in[g][:])
                wq8_sb.append(t)
            load_bias("bq", bq_in, nc.sync)
            for g in range(G):
                t = wpool.tile([128, 2 * C], fp8, tag=f"wk8{g}", name=f"wk8{g}")
                nc.gpsimd.dma_start(out=t[:], in_=wk8_in[g][:])
                wk8_sb.append(t)
            load_bias("bk", bk_in, nc.gpsimd)
            # own-image x8 slices (slot 0) first: 4 small DMAs unblock the q
            # projection; then the remaining images in pair chunks.
            for g in range(G):
                t = xpool.tile([128, 2 * NCORES * HW], fp8, tag=f"x8{g}",
                               name=f"x8{g}")
                x8_sb.append(t)
            for g in range(G):
                for i in range(2):
                    c0 = i * NCORES * HW
                    eng = nc.sync if (g + i) % 2 == 0 else nc.gpsimd
                    eng.dma_start(out=x8_sb[g][:, c0:c0 + HW],
                                  in_=x8_in[g][:, c0:c0 + HW])
            for pair in range(4):
                for g in range(G):
                    for i in range(2):
                        c0 = i * NCORES * HW + max(pair * 2 * HW, HW)
                        c1 = i * NCORES * HW + (pair + 1) * 2 * HW
                        if c1 <= c0:
                            continue
                        eng = nc.sync if (g + i + pair) % 2 == 0 else nc.gpsimd
                        eng.dma_start(out=x8_sb[g][:, c0:c1],
                                      in_=x8_in[g][:, c0:c1])
            # tail-path inputs (not needed until after the image loop)
            x_sb = []
            for ci in range(CB):
                t = xpool.tile([128, HW], bf16, tag=f"x{ci}", name=f"x{ci}")
                nc.sync.dma_start(out=t[:],
                                  in_=x_in[ci * 128:(ci + 1) * 128, :])
                x_sb.append(t)
            w6_sb = []
            for ci in range(CB):
                t = wpool.tile([128, C], bf16, tag=f"w6{ci}", name=f"w6{ci}")
                nc.gpsimd.dma_start(out=t[:],
                                    in_=w6t_in[ci * 128:(ci + 1) * 128, :])
                w6_sb.append(t)
            load_bias("b6", b6_in, nc.gpsimd)

            ones_col = consts.tile([128, 1], f32, tag="ones_col")
            nc.vector.memset(ones_col[:], 1.0)
            ones_row = consts.tile([1, 128], f32, tag="ones_row")
            nc.vector.memset(ones_row[:], 1.0)
            # warm the activation table set at the head so no ACT_TABLE_LOAD
            # lands mid-pipeline (natural_log_exp_and_others covers
            # ln + exp + identity; table-load insertion checks containment
            # against the resident set)
            warm = consts.tile([1, 1], f32, tag="warm")
            if NDVE < QB:
                nc.scalar.activation(warm[:], ones_col[:1, :1], AF.Ln,
                                     bias=0.0, scale=1.0)
            nc.scalar.activation(warm[:], ones_col[:1, :1], AF.Exp,
                                 bias=0.0, scale=1.0)

            # ---- qT in fp8 plane-paired layout: qg[g] [128, 2*HW] ----
            qg = []
            for g in range(G):
                t = qpool.tile([128, G * HW], fp8, tag=f"q{g}", name=f"q{g}")
                qg.append(t)
            for co in range(CB):
                for h in range(KH):
                    ps = ps_s.tile([128, 512], f32, tag="ps_s", name="ps_q")
                    for g in range(G):
                        nc.tensor.matmul(
                            ps[:],
                            dr3(wq8_sb[g][:, :], C)[:, :,
                                                    co * 128:(co + 1) * 128],
                            dr3(x8_sb[g][:, :],
                                NCORES * HW)[:, :, h * 512:(h + 1) * 512],
                            start=(g == 0), stop=(g == G - 1), perf_mode=DR)
                    nc.scalar.activation(
                        qg[co // 2][:, (co % 2) * HW + h * 512:
                                    (co % 2) * HW + (h + 1) * 512],
                        ps[:], AF.Identity, bias=bias_sb["bq"][:, co:co + 1],
                        scale=1.0 / W_SCALE)

            def qg_ap(g, qb):
                return dr3(qg[g][:, :], HW)[:, :, qb * 128:(qb + 1) * 128]

            # mpA/mpB[:, qb*8+img]: per-(query,image) max over key half 0/1
            # (DVE tensor_reduce path); merged by one TT max at the tail.
            if NDVE > 0:
                mpA = redpool.tile([128, NDVE * NIMG], f32, tag="mpA",
                                   name="mpA")
                mpB = redpool.tile([128, NDVE * NIMG], f32, tag="mpB",
                                   name="mpB")
            # sacc[:, (qb-NDVE)*16+img*2+h]: per-half exp-sums (LSE path)
            if NDVE < QB:
                sacc = redpool.tile([128, (QB - NDVE) * NIMG * KH], f32,
                                    tag="sacc", name="sacc")

            # ---- per-image: compute kT locally (fp8 DR), then score ----
            for img in range(NCORES):
                klg = {}
                for h in range(KH):
                    for gd in range(G):
                        klg[(h, gd)] = klpool.tile(
                            [128, G * 512], fp8, tag=f"kl{h}{gd}",
                            name=f"kl{h}{gd}")
                for h in range(KH):
                    for co in range(CB):
                        ps = ps_m.tile([128, 512], f32, tag="ps_m",
                                       name="ps_kf")
                        col0 = img * HW + h * 512
                        for g in range(G):
                            nc.tensor.matmul(
                                ps[:],
                                dr3(wk8_sb[g][:, :], C)[:, :,
                                                        co * 128:(co + 1) * 128],
                                dr3(x8_sb[g][:, :],
                                    NCORES * HW)[:, :, col0:col0 + 512],
                                start=(g == 0), stop=(g == G - 1),
                                perf_mode=DR)
                        nc.scalar.activation(
                            klg[(h, co // 2)][:, (co % 2) * 512:
                                              (co % 2 + 1) * 512],
                            ps[:], AF.Identity,
                            bias=bias_sb["bk"][:, co:co + 1],
                            scale=1.0 / W_SCALE)
                for qb in range(QB):
                    for h in range(KH):
                        ps = ps_s.tile([128, 512], f32, tag="ps_s",
                                       name="ps_sc")
                        for g in range(G):
                            nc.tensor.matmul(
                                ps[:], qg_ap(g, qb),
                                dr3(klg[(h, g)][:, :], 512),
                                start=(g == 0), stop=(g == G - 1),
                                perf_mode=DR)
                        if qb < NDVE:
                            mpt = mpA if h == 0 else mpB
                            col = qb * NIMG + img
                            nc.vector.tensor_reduce(
                                mpt[:, col:col + 1], ps[:],
                                axis=AX.X, op=ALU.max)
                        else:
                            scr = scrpool.tile([128, 512], bf16, tag="scr",
                                               name="scr")
                            col = (qb - NDVE) * NIMG * KH + img * KH + h
                            nc.scalar.activation(
                                scr[:], ps[:], AF.Exp, bias=0.0, scale=LSE_T,
                                accum_out=sacc[:, col:col + 1])

            # ---- softmax over the core's 1024 queries ----
            X8 = redpool.tile([128, QB], f32, tag="X8", name="X8")
            if NDVE > 0:
                mx = redpool.tile([128, NDVE * NIMG], f32, tag="mx",
                                  name="mx")
                nc.vector.tensor_max(mx[:], mpA[:], mpB[:])
                nc.vector.tensor_reduce(
                    X8[:, :NDVE],
                    mx[:, :].rearrange("p (q i) -> p q i", q=NDVE, i=NIMG),
                    axis=AX.X, op=ALU.add)
            if NDVE < QB:
                nq = QB - NDVE
                sh = redpool.tile([128, nq * NIMG], f32, tag="sh", name="sh")
                nc.vector.tensor_reduce(
                    sh[:],
                    sacc[:, :].rearrange("p (n i) -> p n i", n=nq * NIMG,
                                         i=KH),
                    axis=AX.X, op=ALU.add)
                # the HW ln spline is garbage above ~1e19, and the exp-sums
                # reach ~5e21; prescale by 2^-40 (free in the activation)
                # and add the 40*ln2 back per image below.
                lns = redpool.tile([128, nq * NIMG], f32, tag="lns",
                                   name="lns")
                nc.scalar.activation(lns[:], sh[:], AF.Ln, bias=0.0,
                                     scale=2.0 ** -40)
                xl = redpool.tile([128, nq], f32, tag="xl", name="xl")
                nc.vector.tensor_reduce(
                    xl[:],
                    lns[:, :].rearrange("p (q i) -> p q i", q=nq, i=NIMG),
                    axis=AX.X, op=ALU.add)
                # undo the prescale and the LSE temperature
                nc.vector.tensor_scalar(
                    X8[:, NDVE:], xl[:],
                    scalar1=NIMG * 40.0 * float(np.log(2.0)),
                    scalar2=1.0 / LSE_T, op0=ALU.add, op1=ALU.mult)

            # debug dump: sacc (32 cols) + X8 (8 cols) + sh/lns/xl
            if NDVE < QB:
                nc.sync.dma_start(out=dbg_ext[:, 0:(QB - NDVE) * NIMG * KH],
                                  in_=sacc[:, :])
                nc.sync.dma_start(out=dbg_ext[:, 48:64], in_=sh[:, :])
                nc.sync.dma_start(out=dbg_ext[:, 64:80], in_=lns[:, :])
                nc.sync.dma_start(out=dbg_ext[:, 80:82], in_=xl[:, :])
            nc.sync.dma_start(out=dbg_ext[:, 40:40 + QB], in_=X8[:, :])

            # exp without max-subtraction is safe: xw*scale stays ~[0.4,1.2]
            EX = redpool.tile([128, QB], f32, tag="EX", name="EX")
            S1 = redpool.tile([128, 1], f32, tag="S1", name="S1")
            nc.scalar.activation(EX[:], X8[:], AF.Exp, bias=0.0,
                                 scale=SCALE / NIMG, accum_out=S1[:])

            # chain A (reciprocal of the total):
            ps_tot = ps_m.tile([128, 512], f32, tag="ps_m", name="ps_tot")
            nc.tensor.matmul(ps_tot[:1, :1], ones_col[:], S1[:],
                             start=True, stop=True)
            tot = redpool.tile([1, 1], f32, tag="tot", name="tot")
            nc.vector.tensor_copy(out=tot[:], in_=ps_tot[:1, :1])
            rcp = redpool.tile([1, 1], f32, tag="rcp", name="rcp")
            nc.vector.reciprocal(rcp[:], tot[:])
            ps_rb = ps_m.tile([128, 512], f32, tag="ps_m", name="ps_rb")
            nc.tensor.matmul(ps_rb[:, :1], ones_row[:], rcp[:],
                             start=True, stop=True)
            rb = redpool.tile([128, 1], f32, tag="rb", name="rb")
            nc.vector.tensor_copy(out=rb[:], in_=ps_rb[:, :1])

            # chain B (flatten EX across partitions into a [1, 1024] row):
            # bounce through DRAM, read back transposed as 8 column reads
            # spread over the three DMA queues.
            wr_d = dram.tile([128, QB], f32, tag="wr_d", name="wr_d")
            nc.sync.dma_start(out=wr_d[:, :], in_=EX[:, :])
            wrow = redpool.tile([1, HW], f32, tag="wrow", name="wrow")
            qengs = (nc.sync, nc.scalar, nc.gpsimd)
            for qb in range(QB):
                qengs[qb % 3].dma_start(
                    out=wrow[0:1, qb * 128:(qb + 1) * 128],
                    in_=wr_d[:, qb:qb + 1].transpose([1, 0]))

            # broadcast to all partitions via ones[1,128].T @ wrow, folding
            # the 1/total scale into the bf16 PSUM evacuation.
            B_bf = redpool.tile([128, HW], bf16, tag="B_bf", name="B_bf")
            for h in range(KH):
                ps_b = ps_m.tile([128, 512], f32, tag="ps_m", name="ps_b")
                nc.tensor.matmul(ps_b[:], ones_row[:],
                                 wrow[0:1, h * 512:(h + 1) * 512],
                                 start=True, stop=True)
                nc.scalar.activation(B_bf[:, h * 512:(h + 1) * 512],
                                     ps_b[:], AF.Identity, bias=0.0,
                                     scale=rb[:])

            # ---- gate x, then the final 1x1 conv (bf16), f32 out ----
            xg = []
            for ci in range(CB):
                t = qpool.tile([128, HW], bf16, tag=f"xg{ci}", name=f"xg{ci}")
                xg.append(t)
            o_sb = [outpool.tile([128, HW], f32, tag=f"o{co}", name=f"o{co}")
                    for co in range(CB)]
            oengs = (nc.sync, nc.gpsimd)
            for h in range(KH):
                sl = slice(h * 512, (h + 1) * 512)
                for ci in range(CB):
                    nc.vector.tensor_mul(xg[ci][:, sl], x_sb[ci][:, sl],
                                         B_bf[:, sl])
                for co in range(CB):
                    ps = ps_s.tile([128, 512], f32, tag="ps_s", name="ps_y")
                    for ci in range(CB):
                        nc.tensor.matmul(
                            ps[:], w6_sb[ci][:, co * 128:(co + 1) * 128],
                            xg[ci][:, sl],
                            start=(ci == 0), stop=(ci == CB - 1))
                    nc.scalar.activation(o_sb[co][:, sl], ps[:], AF.Identity,
                                         bias=bias_sb["b6"][:, co:co + 1],
                                         scale=1.0)
                    if h == KH - 1:
                        oengs[co % 2].dma_start(
                            out=out_ext[co * 128:(co + 1) * 128, :],
                            in_=o_sb[co][:])

    nc.compile()
    return nc


_BUILT = {}


def _get_nc():
    if "nc" not in _BUILT:
        _BUILT["nc"] = build_kernel()
    return _BUILT["nc"]


def make_in_maps(x, Wq, bq, Wk, bk, W6, b6):
    import ml_dtypes
    e4 = ml_dtypes.float8_e4m3
    bfl = ml_dtypes.bfloat16
    x = np.asarray(x, dtype=np.float32).reshape(B, C, HW)
    w6t = np.ascontiguousarray(np.asarray(W6, np.float32).T).astype(bfl)
    bqc = np.ascontiguousarray(np.asarray(bq, np.float32).reshape(C, 1))
    bkc = np.ascontiguousarray(np.asarray(bk, np.float32).reshape(C, 1))
    b6c = np.ascontiguousarray(np.asarray(b6, np.float32).reshape(C, 1))

    def w_dr(wt):
        """[C,C] -> G x [128, 2*C] fp8 DoubleRow plane-paired layout."""
        ws = (np.asarray(wt, np.float32).T * W_SCALE).astype(e4)
        w8 = ws.reshape(G, 2, 128, C)
        return [np.ascontiguousarray(
            np.transpose(w8[g], (1, 0, 2)).reshape(128, 2 * C))
            for g in range(G)]

    wq8g = w_dr(Wq)
    wk8g = w_dr(Wk)
    # fp8 DoubleRow layouts of x for every core, image-rolled so slot 0 is
    # the core's own image: c = g*256 + i*128 + p
    xc = np.transpose(x, (1, 0, 2))                      # [c, img, hw]
    x8_f = xc.astype(e4)                                 # [C, B, HW]
    maps = []
    for b in range(B):
        order = [(b + i) % B for i in range(B)]
        xr = x8_f[:, order, :].reshape(G, 2, 128, B * HW)
        x8g = [np.ascontiguousarray(
            np.transpose(xr[g], (1, 0, 2)).reshape(128, 2 * B * HW))
            for g in range(G)]
        m = {"x": np.ascontiguousarray(x[b]).astype(bfl),
             "w6t": w6t, "bq": bqc, "bk": bkc, "b6": b6c}
        for g in range(G):
            m[f"x8g{g}"] = x8g[g]
            m[f"wq8g{g}"] = wq8g[g]
            m[f"wk8g{g}"] = wk8g[g]
        maps.append(m)
    return maps


def kernel(x, Wq, bq, Wk, bk, W6, b6, _trace=False):
    from concourse import bass_utils
    nc = _get_nc()
    in_maps = make_in_maps(x, Wq, bq, Wk, bk, W6, b6)
    res = bass_utils.run_bass_kernel_spmd(
        nc, in_maps, core_ids=list(range(NCORES)), trace=_trace)
    out = np.stack([np.asarray(res.results[i]["out"]) for i in range(NCORES)])
    out = out.reshape(B, C, H, W).astype(np.float32)
    if _trace:
        return out, res
    return out


# revision 18
# speedup vs baseline: 1.0698x; 1.0698x over previous
"""Trainium2 Bass kernel for nn_AllAttLayer (cross-batch attention gating layer).

Reference computation (B=8, C=512, H=W=32, HW=1024):
    xf = x as [B, HW, C]
    q = xf @ Wq.T + bq ; k = xf @ Wk.T + bk
    scores = q.flat @ k.flat.T                  # [B*HW, B*HW]
    xw = max over each image's keys, mean over images   # [B*HW]
    xw = softmax(xw * C**-0.5 per image)        # [B, HW]
    out = (x * xw) @ W6.T + b6  (1x1 conv)      # == W6 @ (x * xw)

Sharding: core b owns image b (its 1024 queries). No collectives: the
host replicates the full x (fp8, DoubleRow layout, ROLLED per core so
the core's own image is always slot 0) and scaled fp8 WqT/WkT to every
core; each core computes every image's keys locally with fp8 DoubleRow
projections.

Everything is c-major ([C, HW]: channel on partitions, pixel on free
dim) so PE matmuls need no transposes.

Precision: q/k projections and score matmuls all run fp8e4 DoubleRow
(2 fp8 weights per PE cell -> effective K=256 per matmul, 2x bf16
throughput). Weights are host-scaled by 16 (subnormal avoidance) and
unscaled in the PSUM evacuation. The final conv runs bf16; reductions,
softmax and the output stay fp32.

Engine schedule: every PSUM tile is a PAIRED [128,1024] tile spanning
two banks (matmuls fill the two 512-col halves separately), so each
(query-block, image) needs ONE DVE tensor_reduce (~1.19us) instead of
two (~1.24us) and each key/projection pair ONE scalar evacuation
(~1.0us) instead of two (~1.28us); the per-image max lands directly in
mp (no half-merge pass). The PE instruction stream interleaves the
NEXT image's key projections between the current image's score
matmuls, so the PE keeps running while the DVE drains the score
backlog; per image the PE has ~10.4us of matmuls vs ~9.5us of DVE
reduces and ~4.0us of scalar evacuations. PSUM: 2 score pairs + 2 key
pairs = 8 banks.
"""

import sys
import numpy as np

for _p in ("/opt/trn_rl_repo",):
    if _p not in sys.path:
        sys.path.insert(0, _p)

B, C, H, W = 8, 512, 32, 32
HW = H * W              # 1024 pixels per image
NCORES = 8
CB = C // 128           # 4 channel blocks
G = 2                   # DoubleRow groups (K=256 each)
QB = HW // 128          # 8 query blocks per core
KH = 2                  # key halves (512 keys each)
NIMG = NCORES
SCALE = 1.0 / float(np.sqrt(C))

W_SCALE = 16.0          # host scales WqT/WkT by this before fp8
DEBUG = False


def build_kernel():
    from concourse import bacc, tile, mybir

    f32 = mybir.dt.float32
    bf16 = mybir.dt.bfloat16
    fp8 = mybir.dt.float8e4
    DR = mybir.MatmulPerfMode.DoubleRow

    nc = bacc.Bacc("TRN2", target_bir_lowering=False, debug=False,
                   num_devices=NCORES)

    # per-core own image (bf16, for the final conv) and weights
    x_in = nc.dram_tensor("x", [C, HW], bf16, kind="ExternalInput").ap()
    w6t_in = nc.dram_tensor("w6t", [C, C], bf16, kind="ExternalInput").ap()
    # replicated full x (rolled: slot 0 = own image) and scaled WqT/WkT in
    # fp8 DoubleRow layouts
    x8_in = [nc.dram_tensor(f"x8g{g}", [128, 2 * NCORES * HW], fp8,
                            kind="ExternalInput").ap() for g in range(G)]
    wq8_in = [nc.dram_tensor(f"wq8g{g}", [128, 2 * C], fp8,
                             kind="ExternalInput").ap() for g in range(G)]
    wk8_in = [nc.dram_tensor(f"wk8g{g}", [128, 2 * C], fp8,
                             kind="ExternalInput").ap() for g in range(G)]
    bq_in = nc.dram_tensor("bq", [C, 1], f32, kind="ExternalInput").ap()
    bk_in = nc.dram_tensor("bk", [C, 1], f32, kind="ExternalInput").ap()
    b6_in = nc.dram_tensor("b6", [C, 1], f32, kind="ExternalInput").ap()
    out_ext = nc.dram_tensor("out", [C, HW], f32, kind="ExternalOutput").ap()
    if DEBUG:
        dbg_ext = nc.dram_tensor("dbg", [128, 128], f32,
                                 kind="ExternalOutput").ap()

    AF = mybir.ActivationFunctionType
    ALU = mybir.AluOpType
    AX = mybir.AxisListType

    def dr3(ap, span):
        """[128, G*span] tile AP -> [128, 2, span] DoubleRow view."""
        return ap.rearrange("p (i n) -> p i n", i=2, n=span)

    with tile.TileContext(nc) as tc:
        with tc.tile_pool(name="consts", bufs=1) as consts, \
             tc.tile_pool(name="wpool", bufs=1) as wpool, \
             tc.tile_pool(name="xpool", bufs=1) as xpool, \
             tc.tile_pool(name="qpool", bufs=1) as qpool, \
             tc.tile_pool(name="klpool", bufs=2) as klpool, \
             tc.tile_pool(name="redpool", bufs=1) as redpool, \
             tc.tile_pool(name="outpool", bufs=1) as outpool, \
             tc.tile_pool(name="dram", bufs=1, space="DRAM") as dram, \
             tc.tile_pool(name="ps_s", bufs=2, space="PSUM") as ps_s, \
             tc.tile_pool(name="ps_k", bufs=2, space="PSUM") as ps_k:

            bias_sb = {}

            def load_bias(nm, src, eng):
                t = consts.tile([128, CB], f32, tag=f"{nm}_sb", name=f"{nm}_sb")
                for co in range(CB):
                    eng.dma_start(out=t[:, co:co + 1],
                                  in_=src[co * 128:(co + 1) * 128, :])
                bias_sb[nm] = t

            # ---- head loads ----
            # Own-image x8 slices (slot 0) and the q/k weights first so the
            # first projections aren't starved; the scalar queue gets no
            # head DMAs (its engine time is needed for evacuations).
            x8_sb = []
            for g in range(G):
                t = xpool.tile([128, 2 * NCORES * HW], fp8, tag=f"x8{g}",
                               name=f"x8{g}")
                x8_sb.append(t)
            wq8_sb, wk8_sb = [], []
            for g in range(G):
                t = wpool.tile([128, 2 * C], fp8, tag=f"wq8{g}", name=f"wq8{g}")
                nc.sync.dma_start(out=t[:], in_=wq8_in[g][:])
                wq8_sb.append(t)
            for g in range(G):
                for i in range(2):
                    c0 = i * NCORES * HW
                    eng = nc.sync if (g + i) % 2 == 0 else nc.gpsimd
                    eng.dma_start(out=x8_sb[g][:, c0:c0 + HW],
                                  in_=x8_in[g][:, c0:c0 + HW])
            for g in range(G):
                t = wpool.tile([128, 2 * C], fp8, tag=f"wk8{g}", name=f"wk8{g}")
                nc.gpsimd.dma_start(out=t[:], in_=wk8_in[g][:])
                wk8_sb.append(t)
            load_bias("bq", bq_in, nc.sync)
            load_bias("bk", bk_in, nc.gpsimd)
            for pair in range(4):
                for g in range(G):
                    for i in range(2):
                        c0 = i * NCORES * HW + max(pair * 2 * HW, HW)
                        c1 = i * NCORES * HW + (pair + 1) * 2 * HW
                        if c1 <= c0:
                            continue
                        eng = nc.sync if (g + i + pair) % 2 == 0 else nc.gpsimd
                        eng.dma_start(out=x8_sb[g][:, c0:c1],
                                      in_=x8_in[g][:, c0:c1])
            # tail-path inputs (not needed until after the image loop)
            x_sb = []
            for ci in range(CB):
                t = xpool.tile([128, HW], bf16, tag=f"x{ci}", name=f"x{ci}")
                nc.sync.dma_start(out=t[:],
                                  in_=x_in[ci * 128:(ci + 1) * 128, :])
                x_sb.append(t)
            w6_sb = []
            for ci in range(CB):
                t = wpool.tile([128, C], bf16, tag=f"w6{ci}", name=f"w6{ci}")
                nc.gpsimd.dma_start(out=t[:],
                                    in_=w6t_in[ci * 128:(ci + 1) * 128, :])
                w6_sb.append(t)
            load_bias("b6", b6_in, nc.gpsimd)

            ones_col = consts.tile([128, 1], f32, tag="ones_col")
            nc.vector.memset(ones_col[:], 1.0)
            ones_row = consts.tile([1, 128], f32, tag="ones_row")
            nc.vector.memset(ones_row[:], 1.0)
            ones_row_bf = consts.tile([1, 128], bf16, tag="ones_row_bf")
            nc.vector.memset(ones_row_bf[:], 1.0)
            # warm the exp table set at the head (exp_and_others also holds
            # identity) so the tail softmax pays no ACT_TABLE_LOAD
            warm = consts.tile([1, 1], f32, tag="warm")
            nc.scalar.activation(warm[:], ones_col[:1, :1], AF.Exp,
                                 bias=0.0, scale=1.0)

            # ---- q projection (fp8 DR) into qg[g] [128, 2*HW] ----
            # one paired [128,1024] PSUM tile per co (both key halves),
            # evacuated by a single scalar activation.
            qg = [qpool.tile([128, G * HW], fp8, tag=f"q{g}", name=f"q{g}")
                  for g in range(G)]

            def q_pair(co):
                ps = ps_s.tile([128, 1024], f32, tag="ps_s", name="ps_q")
                for h in range(KH):
                    for g in range(G):
                        nc.tensor.matmul(
                            ps[:, h * 512:(h + 1) * 512],
                            dr3(wq8_sb[g][:, :], C)[:, :,
                                                    co * 128:(co + 1) * 128],
                            dr3(x8_sb[g][:, :],
                                NCORES * HW)[:, :, h * 512:(h + 1) * 512],
                            start=(g == 0), stop=(g == G - 1), perf_mode=DR)
                nc.scalar.activation(
                    qg[co // 2][:, (co % 2) * HW:(co % 2 + 1) * HW],
                    ps[:], AF.Identity, bias=bias_sb["bq"][:, co:co + 1],
                    scale=1.0 / W_SCALE)

            def qg_ap(g, qb):
                return dr3(qg[g][:, :], HW)[:, :, qb * 128:(qb + 1) * 128]

            # klg2[g]: [128, 2048] fp8, layout [p, i*1024 + h*512 + key]
            def k_pair(kl, img, co):
                """project keys of `img` for channel block co (both halves)
                into half i=co%2 of the caller-provided klg2[g] tile."""
                i = co % 2
                ps = ps_k.tile([128, 1024], f32, tag="ps_k", name="ps_kf")
                for h in range(KH):
                    col0 = img * HW + h * 512
                    for gg in range(G):
                        nc.tensor.matmul(
                            ps[:, h * 512:(h + 1) * 512],
                            dr3(wk8_sb[gg][:, :], C)[:, :,
                                                     co * 128:(co + 1) * 128],
                            dr3(x8_sb[gg][:, :],
                                NCORES * HW)[:, :, col0:col0 + 512],
                            start=(gg == 0), stop=(gg == G - 1), perf_mode=DR)
                nc.scalar.activation(
                    kl[:, i * 1024:(i + 1) * 1024], ps[:], AF.Identity,
                    bias=bias_sb["bk"][:, co:co + 1], scale=1.0 / W_SCALE)

            # mp[:, qb*8+img]: per-(query,image) max
            mp = redpool.tile([128, QB * NIMG], f32, tag="mp", name="mp")

            def score_pair(klg2, qb, img):
                """both 512-key halves of (qb, img) -> one [128,1024] PSUM
                pair -> one DVE max-reduce into mp."""
                ps = ps_s.tile([128, 1024], f32, tag="ps_s", name="ps_sc")
                for h in range(KH):
                    for g in range(G):
                        rhs = klg2[g][:, :].rearrange(
                            "p (i n) -> p i n", i=2,
                            n=1024)[:, :, h * 512:(h + 1) * 512]
                        nc.tensor.matmul(
                            ps[:, h * 512:(h + 1) * 512], qg_ap(g, qb), rhs,
                            start=(g == 0), stop=(g == G - 1), perf_mode=DR)
                col = qb * NIMG + img
                nc.vector.tensor_reduce(
                    mp[:, col:col + 1], ps[:], axis=AX.X, op=ALU.max)

            # ---- pipelined q + key + score schedule ----
            # q pairs and image-0 key pairs interleave at the head; then for
            # each image the NEXT image's key pairs are emitted between
            # score pairs so the PE never waits on the DVE's score backlog.
            def alloc_klg2():
                return [klpool.tile([128, 2 * 1024], fp8, tag=f"kl{g}",
                                    name=f"kl{g}") for g in range(G)]

            klg2 = alloc_klg2()
            for co in range(CB):
                q_pair(co)
                k_pair(klg2[co // 2], 0, co)
            for img in range(NIMG):
                nxtk = alloc_klg2() if img + 1 < NIMG else None
                for qb in range(QB):
                    score_pair(klg2, qb, img)
                    if nxtk is not None and qb % 2 == 1:
                        co = qb // 2
                        k_pair(nxtk[co // 2], img + 1, co)
                klg2 = nxtk

            # ---- softmax over the core's 1024 queries ----
            X8 = redpool.tile([128, QB], f32, tag="X8", name="X8")
            nc.vector.tensor_reduce(
                X8[:, :],
                mp[:, :].rearrange("p (q i) -> p q i", q=QB, i=NIMG),
                axis=AX.X, op=ALU.add)

            # exp without max-subtraction is safe: xw*scale stays ~[0.4,1.2]
            EX = redpool.tile([128, QB], f32, tag="EX", name="EX")
            S1 = redpool.tile([128, 1], f32, tag="S1", name="S1")
            nc.scalar.activation(EX[:], X8[:], AF.Exp, bias=0.0,
                                 scale=SCALE / NIMG, accum_out=S1[:])

            # chain A (reciprocal of the total):
            ps_tot = ps_k.tile([128, 1024], f32, tag="ps_k", name="ps_tot")
            nc.tensor.matmul(ps_tot[:1, :1], ones_col[:], S1[:],
                             start=True, stop=True)
            tot = redpool.tile([1, 1], f32, tag="tot", name="tot")
            nc.vector.tensor_copy(out=tot[:], in_=ps_tot[:1, :1])
            rcp = redpool.tile([1, 1], f32, tag="rcp", name="rcp")
            nc.vector.reciprocal(rcp[:], tot[:])
            ps_rb = ps_k.tile([128, 1024], f32, tag="ps_k", name="ps_rb")
            nc.tensor.matmul(ps_rb[:, :1], ones_row[:], rcp[:],
                             start=True, stop=True)
            rb = redpool.tile([128, 1], f32, tag="rb", name="rb")
            nc.vector.tensor_copy(out=rb[:], in_=ps_rb[:, :1])

            # chain B (flatten EX across partitions into a [1, 1024] row):
            # bounce through DRAM, read back transposed as 8 column reads
            # spread over the three DMA queues.
            wr_d = dram.tile([128, QB], f32, tag="wr_d", name="wr_d")
            nc.sync.dma_start(out=wr_d[:, :], in_=EX[:, :])
            wrow = redpool.tile([1, HW], f32, tag="wrow", name="wrow")
            qengs = (nc.sync, nc.scalar, nc.gpsimd)
            for qb in range(QB):
                qengs[qb % 3].dma_start(
                    out=wrow[0:1, qb * 128:(qb + 1) * 128],
                    in_=wr_d[:, qb:qb + 1].transpose([1, 0]))
            wrow_bf = redpool.tile([1, HW], bf16, tag="wrow_bf",
                                   name="wrow_bf")
            nc.vector.tensor_copy(out=wrow_bf[:], in_=wrow[:])

            # broadcast to all partitions via ones[1,128].T @ wrow (bf16,
            # 1 cyc/row), folding the 1/total scale into the bf16 PSUM
            # evacuation of the paired tile.
            B_bf = redpool.tile([128, HW], bf16, tag="B_bf", name="B_bf")
            ps_b = ps_s.tile([128, 1024], f32, tag="ps_s", name="ps_b")
            for h in range(KH):
                nc.tensor.matmul(ps_b[:, h * 512:(h + 1) * 512],
                                 ones_row_bf[:],
                                 wrow_bf[0:1, h * 512:(h + 1) * 512],
                                 start=True, stop=True)
            nc.scalar.activation(B_bf[:, :], ps_b[:], AF.Identity, bias=0.0,
                                 scale=rb[:])

            # ---- gate x, then the final 1x1 conv (bf16), f32 out ----
            xg = []
            for ci in range(CB):
                t = qpool.tile([128, HW], bf16, tag=f"xg{ci}", name=f"xg{ci}")
                nc.vector.tensor_mul(t[:], x_sb[ci][:], B_bf[:])
                xg.append(t)
            oengs = (nc.sync, nc.gpsimd)
            for co in range(CB):
                o = outpool.tile([128, HW], f32, tag=f"o{co}", name=f"o{co}")
                ps = ps_s.tile([128, 1024], f32, tag="ps_s", name="ps_y")
                for h in range(KH):
                    sl = slice(h * 512, (h + 1) * 512)
                    for ci in range(CB):
                        nc.tensor.matmul(
                            ps[:, sl], w6_sb[ci][:, co * 128:(co + 1) * 128],
                            xg[ci][:, sl],
                            start=(ci == 0), stop=(ci == CB - 1))
                nc.scalar.activation(o[:], ps[:], AF.Identity,
                                     bias=bias_sb["b6"][:, co:co + 1],
                                     scale=1.0)
                oengs[co % 2].dma_start(
                    out=out_ext[co * 128:(co + 1) * 128, :], in_=o[:])

    nc.compile()
    return nc


_BUILT = {}


def _get_nc():
    if "nc" not in _BUILT:
        _BUILT["nc"] = build_kernel()
    return _BUILT["nc"]


def make_in_maps(x, Wq, bq, Wk, bk, W6, b6):
    import ml_dtypes
    e4 = ml_dtypes.float8_e4m3
    bfl = ml_dtypes.bfloat16
    x = np.asarray(x, dtype=np.float32).reshape(B, C, HW)
    w6t = np.ascontiguousarray(np.asarray(W6, np.float32).T).astype(bfl)
    bqc = np.ascontiguousarray(np.asarray(bq, np.float32).reshape(C, 1))
    bkc = np.ascontiguousarray(np.asarray(bk, np.float32).reshape(C, 1))
    b6c = np.ascontiguousarray(np.asarray(b6, np.float32).reshape(C, 1))

    def w_dr(wt):
        """[C,C] -> G x [128, 2*C] fp8 DoubleRow plane-paired layout."""
        ws = (np.asarray(wt, np.float32).T * W_SCALE).astype(e4)
        w8 = ws.reshape(G, 2, 128, C)
        return [np.ascontiguousarray(
            np.transpose(w8[g], (1, 0, 2)).reshape(128, 2 * C))
            for g in range(G)]

    wq8g = w_dr(Wq)
    wk8g = w_dr(Wk)
    # fp8 DoubleRow layouts of x for every core, image-rolled so slot 0 is
    # the core's own image: c = g*256 + i*128 + p
    xc = np.transpose(x, (1, 0, 2))                      # [c, img, hw]
    x8_f = xc.astype(e4)                                 # [C, B, HW]
    maps = []
    for b in range(B):
        order = [(b + i) % B for i in range(B)]
        xr = x8_f[:, order, :].reshape(G, 2, 128, B * HW)
        x8g = [np.ascontiguousarray(
            np.transpose(xr[g], (1, 0, 2)).reshape(128, 2 * B * HW))
            for g in range(G)]
        m = {"x": np.ascontiguousarray(x[b]).astype(bfl),
             "w6t": w6t, "bq": bqc, "bk": bkc, "b6": b6c}
        for g in range(G):
            m[f"x8g{g}"] = x8g[g]
            m[f"wq8g{g}"] = wq8g[g]
            m[f"wk8g{g}"] = wk8g[g]
        maps.append(m)
    return maps


def kernel(x, Wq, bq, Wk, bk, W6, b6, _trace=False):
    from concourse import bass_utils
    nc = _get_nc()
    in_maps = make_in_maps(x, Wq, bq, Wk, bk, W6, b6)
    res = bass_utils.run_bass_kernel_spmd(
        nc, in_maps, core_ids=list(range(NCORES)), trace=_trace)
    out = np.stack([np.asarray(res.results[i]["out"]) for i in range(NCORES)])
    out = out.reshape(B, C, H, W).astype(np.float32)
    if _trace:
        return out, res
    return out


# revision 23
# speedup vs baseline: 1.3346x; 1.2475x over previous
"""Trainium2 Bass kernel for nn_AllAttLayer (cross-batch attention gating layer).

Reference computation (B=8, C=512, H=W=32, HW=1024):
    xf = x as [B, HW, C]
    q = xf @ Wq.T + bq ; k = xf @ Wk.T + bk
    scores = q.flat @ k.flat.T                  # [B*HW, B*HW]
    xw = max over each image's keys, mean over images   # [B*HW]
    xw = softmax(xw * C**-0.5 per image)        # [B, HW]
    out = (x * xw) @ W6.T + b6  (1x1 conv)      # == W6 @ (x * xw)

Key algebraic restructure: scores = q^T k = q^T (Wk x + bk)
  = (Wk^T Wq x + Wk^T bq)^T x  +  (x^T Wq^T bk + bq^T bk).
The host folds Wqk := Wk^T Wq and bqk := Wk^T bq, so the kernel projects
each core's queries ONCE (qt = Wqk x + bqk, fp8 DoubleRow from the
replicated fp8 x) and the score matmuls consume the replicated fp8 x
DIRECTLY as the moving operand -- no per-image key projection, no key
evacuations. The k-bias term x^T(Wq^T bk) =: qbk is a 1-column fp8
projection (host-folded hbk := Wq^T bk) added to the logits; its
constant part bq^T bk is uniform over all queries and cancels in
softmax.

Sharding: core b owns image b (its 1024 queries). No collectives: the
host replicates x in fp8 DoubleRow layout, ROLLED per core so the
core's own image is slot 0 (the kernel is SPMD -- same program, per-core
data). Everything is c-major ([C, HW]) so no transposes are needed.

Engine schedule: per (query-block, image) the two 512-key score halves
land in one paired [128,1024] PSUM tile (2 banks); query blocks 0..NDVE-1
are consumed by a single DVE max-reduce (~1.19us), the rest by a
ScalarE exp-accumulate (LSE max approximation with temperature 2 and a
-80 shift to keep exp sums in range; the ~ln(n_eff)/2 overestimate is
~0.5 on logits*SCALE/8 ~ 0.003, well under the tolerance). Per image:
PE 7.6us of score matmuls vs DVE ~6.0us + scalar ~3.5us -- PE-paced.
The final conv runs bf16 on ungated x DURING the last image's score
drain; the gate (and b6) applies at the tail. fp32 elsewhere.
"""

import sys
import numpy as np

for _p in ("/opt/trn_rl_repo",):
    if _p not in sys.path:
        sys.path.insert(0, _p)

B, C, H, W = 8, 512, 32, 32
HW = H * W              # 1024 pixels per image
NCORES = 8
CB = C // 128           # 4 channel blocks
G = 2                   # DoubleRow groups (K=256 each)
QB = HW // 128          # 8 query blocks per core
KH = 2                  # key halves (512 keys each)
NIMG = NCORES
SCALE = 1.0 / float(np.sqrt(C))

WQK_SCALE = 64.0        # host scales Wqk by this before fp8
HBK_SCALE = 16.0        # host scales hbk by this before fp8
NDVE = 5                # query blocks per image consumed by DVE (rest: LSE)
LSE_T = 2.0             # LSE temperature
LSE_SHIFT = 36.0        # exp(t*s-SHIFT): sums stay in the HW ln
                        # spline's valid range [1e-18, 1e19]


def build_kernel():
    from concourse import bacc, tile, mybir

    f32 = mybir.dt.float32
    bf16 = mybir.dt.bfloat16
    fp8 = mybir.dt.float8e4
    DR = mybir.MatmulPerfMode.DoubleRow

    nc = bacc.Bacc("TRN2", target_bir_lowering=False, debug=False,
                   num_devices=NCORES)

    x_in = nc.dram_tensor("x", [C, HW], bf16, kind="ExternalInput").ap()
    w6t_in = nc.dram_tensor("w6t", [C, C], bf16, kind="ExternalInput").ap()
    x8_in = [nc.dram_tensor(f"x8g{g}", [128, 2 * NCORES * HW], fp8,
                            kind="ExternalInput").ap() for g in range(G)]
    wqk8_in = [nc.dram_tensor(f"wqk8g{g}", [128, 2 * C], fp8,
                              kind="ExternalInput").ap() for g in range(G)]
    hbk8_in = [nc.dram_tensor(f"hbk8g{g}", [128, 2], fp8,
                              kind="ExternalInput").ap() for g in range(G)]
    bqk_in = nc.dram_tensor("bqk", [C, 1], f32, kind="ExternalInput").ap()
    b6_in = nc.dram_tensor("b6", [C, 1], f32, kind="ExternalInput").ap()
    out_ext = nc.dram_tensor("out", [C, HW], f32, kind="ExternalOutput").ap()

    AF = mybir.ActivationFunctionType
    ALU = mybir.AluOpType
    AX = mybir.AxisListType

    def dr3(ap, span):
        """[128, G*span] tile AP -> [128, 2, span] DoubleRow view."""
        return ap.rearrange("p (i n) -> p i n", i=2, n=span)

    with tile.TileContext(nc) as tc:
        with tc.tile_pool(name="consts", bufs=1) as consts, \
             tc.tile_pool(name="wpool", bufs=1) as wpool, \
             tc.tile_pool(name="xpool", bufs=1) as xpool, \
             tc.tile_pool(name="qpool", bufs=1) as qpool, \
             tc.tile_pool(name="redpool", bufs=1) as redpool, \
             tc.tile_pool(name="scrpool", bufs=3) as scrpool, \
             tc.tile_pool(name="outpool", bufs=1) as outpool, \
             tc.tile_pool(name="dram", bufs=1, space="DRAM") as dram, \
             tc.tile_pool(name="ps", bufs=4, space="PSUM") as psp:

            bias_sb = {}

            def load_bias(nm, src, eng):
                t = consts.tile([128, CB], f32, tag=f"{nm}_sb", name=f"{nm}_sb")
                for co in range(CB):
                    eng.dma_start(out=t[:, co:co + 1],
                                  in_=src[co * 128:(co + 1) * 128, :])
                bias_sb[nm] = t

            # ---- head loads ----
            # own-image x8 slices (slot 0) and Wqk first (they gate the
            # qt projection); the scalar queue gets no head DMAs.
            x8_sb = []
            for g in range(G):
                t = xpool.tile([128, 2 * NCORES * HW], fp8, tag=f"x8{g}",
                               name=f"x8{g}")
                x8_sb.append(t)
            wqk8_sb, hbk8_sb = [], []
            for g in range(G):
                t = wpool.tile([128, 2 * C], fp8, tag=f"wqk8{g}",
                               name=f"wqk8{g}")
                nc.sync.dma_start(out=t[:], in_=wqk8_in[g][:])
                wqk8_sb.append(t)
            for g in range(G):
                for i in range(2):
                    c0 = i * NCORES * HW
                    eng = nc.sync if (g + i) % 2 == 0 else nc.gpsimd
                    eng.dma_start(out=x8_sb[g][:, c0:c0 + HW],
                                  in_=x8_in[g][:, c0:c0 + HW])
            load_bias("bqk", bqk_in, nc.sync)
            for g in range(G):
                t = wpool.tile([128, 2], fp8, tag=f"hbk8{g}", name=f"hbk8{g}")
                nc.gpsimd.dma_start(out=t[:], in_=hbk8_in[g][:])
                hbk8_sb.append(t)
            for pair in range(4):
                for g in range(G):
                    for i in range(2):
                        c0 = i * NCORES * HW + max(pair * 2 * HW, HW)
                        c1 = i * NCORES * HW + (pair + 1) * 2 * HW
                        if c1 <= c0:
                            continue
                        eng = nc.sync if (g + i + pair) % 2 == 0 else nc.gpsimd
                        eng.dma_start(out=x8_sb[g][:, c0:c1],
                                      in_=x8_in[g][:, c0:c1])
            # tail-path inputs
            x_sb = []
            for ci in range(CB):
                t = xpool.tile([128, HW], bf16, tag=f"x{ci}", name=f"x{ci}")
                nc.sync.dma_start(out=t[:],
                                  in_=x_in[ci * 128:(ci + 1) * 128, :])
                x_sb.append(t)
            w6_sb = []
            for ci in range(CB):
                t = wpool.tile([128, C], bf16, tag=f"w6{ci}", name=f"w6{ci}")
                nc.gpsimd.dma_start(out=t[:],
                                    in_=w6t_in[ci * 128:(ci + 1) * 128, :])
                w6_sb.append(t)
            load_bias("b6", b6_in, nc.gpsimd)

            ones_col = consts.tile([128, 1], f32, tag="ones_col")
            nc.vector.memset(ones_col[:], 1.0)
            ones_row = consts.tile([1, 128], f32, tag="ones_row")
            nc.vector.memset(ones_row[:], 1.0)
            ones_row_bf = consts.tile([1, 128], bf16, tag="ones_row_bf")
            nc.vector.memset(ones_row_bf[:], 1.0)
            # warm the ln+exp+identity table set so no ACT_TABLE_LOAD lands
            # mid-pipeline (containment check keeps it resident)
            warm = consts.tile([1, 1], f32, tag="warm")
            nc.scalar.activation(warm[:], ones_col[:1, :1], AF.Ln,
                                 bias=0.0, scale=1.0)
            nc.scalar.activation(warm[:], ones_col[:1, :1], AF.Exp,
                                 bias=0.0, scale=1.0)
            shift_col = consts.tile([128, 1], f32, tag="shift_col")
            nc.vector.memset(shift_col[:], -LSE_SHIFT)

            # ---- qt = Wqk @ x_own + bqk, fp8 DR, plane-paired over c ----
            qt = [qpool.tile([128, G * HW], fp8, tag=f"qt{g}", name=f"qt{g}")
                  for g in range(G)]
            for co in range(CB):
                g, i = co // 2, co % 2
                ps = psp.tile([128, 1024], f32, tag="ps", name="ps_qt")
                for h in range(KH):
                    for gg in range(G):
                        nc.tensor.matmul(
                            ps[:, h * 512:(h + 1) * 512],
                            dr3(wqk8_sb[gg][:, :], C)[:, :,
                                                      co * 128:(co + 1) * 128],
                            dr3(x8_sb[gg][:, :],
                                NCORES * HW)[:, :, h * 512:(h + 1) * 512],
                            start=(gg == 0), stop=(gg == G - 1), perf_mode=DR)
                nc.scalar.activation(
                    qt[g][:, i * HW:(i + 1) * HW], ps[:], AF.Identity,
                    bias=bias_sb["bqk"][:, co:co + 1], scale=1.0 / WQK_SCALE)

            def qt_ap(g, qb):
                return dr3(qt[g][:, :], HW)[:, :, qb * 128:(qb + 1) * 128]

            # per-(query,image) true max (DVE path), qb < NDVE
            mp = redpool.tile([128, NDVE * NIMG], f32, tag="mp", name="mp")
            # per-(query-block,image) shifted exp sums (LSE path)
            sacc = redpool.tile([128, (QB - NDVE) * NIMG], f32, tag="sacc",
                                name="sacc")
            # qbk partials accumulate here across the image loop
            ps_qbk = psp.tile([128, 1024], f32, tag="ps", name="ps_qbk")

            # ---- image loop: 8 paired score tiles each ----
            for img in range(NIMG):
                for qb in range(QB):
                    ps = psp.tile([128, 1024], f32, tag="ps", name="ps_sc")
                    for h in range(KH):
                        col0 = img * HW + h * 512
                        for gg in range(G):
                            nc.tensor.matmul(
                                ps[:, h * 512:(h + 1) * 512], qt_ap(gg, qb),
                                dr3(x8_sb[gg][:, :],
                                    NCORES * HW)[:, :, col0:col0 + 512],
                                start=(gg == 0), stop=(gg == G - 1),
                                perf_mode=DR)
                    if qb < NDVE:
                        col = qb * NIMG + img
                        nc.vector.tensor_reduce(
                            mp[:, col:col + 1], ps[:], axis=AX.X, op=ALU.max)
                    else:
                        scr = scrpool.tile([128, 1024], bf16, tag="scr",
                                           name="scr")
                        col = (qb - NDVE) * NIMG + img
                        nc.scalar.activation(
                            scr[:], ps[:], AF.Exp, bias=shift_col[:],
                            scale=LSE_T, accum_out=sacc[:, col:col + 1])
                # qbk partials: 1 query block per image (tiny, hides in loop)
                for gg in range(G):
                    nc.tensor.matmul(
                        ps_qbk[:, img:img + 1],
                        dr3(x8_sb[gg][:, :],
                            NCORES * HW)[:, :, img * 128:(img + 1) * 128],
                        dr3(hbk8_sb[gg][:, :], 1),
                        start=(gg == 0), stop=(gg == G - 1), perf_mode=DR)

            # ---- final conv on UNGATED x (bf16) during the img7 drain ----
            y_sb = [qpool.tile([128, HW], bf16, tag=f"y{co}", name=f"y{co}")
                    for co in range(CB)]
            for co in range(CB):
                ps = psp.tile([128, 1024], f32, tag="ps", name="ps_y")
                for h in range(KH):
                    sl = slice(h * 512, (h + 1) * 512)
                    for ci in range(CB):
                        nc.tensor.matmul(
                            ps[:, sl], w6_sb[ci][:, co * 128:(co + 1) * 128],
                            x_sb[ci][:, sl],
                            start=(ci == 0), stop=(ci == CB - 1))
                nc.scalar.activation(y_sb[co][:], ps[:], AF.Identity,
                                     bias=0.0, scale=1.0)

            # qbk evacuation (x8^T hbk * NIMG; the mean's /NIMG is folded
            # into the exp scale, so multiply by NIMG here)
            qbk_sb = redpool.tile([128, QB], f32, tag="qbk", name="qbk")
            nc.scalar.activation(qbk_sb[:], ps_qbk[:, :QB], AF.Identity,
                                 bias=0.0, scale=float(NIMG) / HBK_SCALE)

            # ---- softmax over the core's 1024 queries ----
            X8 = redpool.tile([128, QB], f32, tag="X8", name="X8")
            nc.vector.tensor_reduce(
                X8[:, :NDVE],
                mp[:, :].rearrange("p (q i) -> p q i", q=NDVE, i=NIMG),
                axis=AX.X, op=ALU.add)
            nq = QB - NDVE
            lns = redpool.tile([128, nq * NIMG], f32, tag="lns", name="lns")
            nc.scalar.activation(lns[:], sacc[:], AF.Ln, bias=0.0, scale=1.0)
            xl = redpool.tile([128, nq], f32, tag="xl", name="xl")
            nc.vector.tensor_reduce(
                xl[:], lns[:, :].rearrange("p (q i) -> p q i", q=nq, i=NIMG),
                axis=AX.X, op=ALU.add)
            nc.vector.tensor_scalar(
                X8[:, NDVE:], xl[:], scalar1=NIMG * LSE_SHIFT,
                scalar2=1.0 / LSE_T, op0=ALU.add, op1=ALU.mult)
            X8b = redpool.tile([128, QB], f32, tag="X8b", name="X8b")
            nc.vector.tensor_add(X8b[:], X8[:], qbk_sb[:])

            EX = redpool.tile([128, QB], f32, tag="EX", name="EX")
            S1 = redpool.tile([128, 1], f32, tag="S1", name="S1")
            nc.scalar.activation(EX[:], X8b[:], AF.Exp, bias=0.0,
                                 scale=SCALE / NIMG, accum_out=S1[:])

            # chain A (reciprocal of the total):
            ps_tot = psp.tile([128, 1024], f32, tag="ps", name="ps_tot")
            nc.tensor.matmul(ps_tot[:1, :1], ones_col[:], S1[:],
                             start=True, stop=True)
            tot = redpool.tile([1, 1], f32, tag="tot", name="tot")
            nc.vector.tensor_copy(out=tot[:], in_=ps_tot[:1, :1])
            rcp = redpool.tile([1, 1], f32, tag="rcp", name="rcp")
            nc.vector.reciprocal(rcp[:], tot[:])
            ps_rb = psp.tile([128, 1024], f32, tag="ps", name="ps_rb")
            nc.tensor.matmul(ps_rb[:, :1], ones_row[:], rcp[:],
                             start=True, stop=True)
            rb = redpool.tile([128, 1], f32, tag="rb", name="rb")
            nc.vector.tensor_copy(out=rb[:], in_=ps_rb[:, :1])

            # chain B (flatten EX across partitions into a [1,1024] row):
            wr_d = dram.tile([128, QB], f32, tag="wr_d", name="wr_d")
            nc.sync.dma_start(out=wr_d[:, :], in_=EX[:, :])
            wrow = redpool.tile([1, HW], f32, tag="wrow", name="wrow")
            qengs = (nc.sync, nc.scalar, nc.gpsimd)
            for qb in range(QB):
                qengs[qb % 3].dma_start(
                    out=wrow[0:1, qb * 128:(qb + 1) * 128],
                    in_=wr_d[:, qb:qb + 1].transpose([1, 0]))
            wrow_bf = redpool.tile([1, HW], bf16, tag="wrow_bf",
                                   name="wrow_bf")
            nc.vector.tensor_copy(out=wrow_bf[:], in_=wrow[:])

            # broadcast to all partitions (bf16 matmul), fold 1/total into
            # the bf16 evacuation
            B_bf = redpool.tile([128, HW], bf16, tag="B_bf", name="B_bf")
            ps_b = psp.tile([128, 1024], f32, tag="ps", name="ps_b")
            for h in range(KH):
                nc.tensor.matmul(ps_b[:, h * 512:(h + 1) * 512],
                                 ones_row_bf[:],
                                 wrow_bf[0:1, h * 512:(h + 1) * 512],
                                 start=True, stop=True)
            nc.scalar.activation(B_bf[:, :], ps_b[:], AF.Identity, bias=0.0,
                                 scale=rb[:])

            # ---- gate y, add b6, write f32 out ----
            oengs = (nc.sync, nc.gpsimd)
            for co in range(CB):
                og = qpool.tile([128, HW], bf16, tag=f"og{co}",
                                name=f"og{co}")
                nc.vector.tensor_mul(og[:], y_sb[co][:], B_bf[:])
                o = outpool.tile([128, HW], f32, tag=f"o{co}", name=f"o{co}")
                nc.scalar.activation(o[:], og[:], AF.Identity,
                                     bias=bias_sb["b6"][:, co:co + 1],
                                     scale=1.0)
                oengs[co % 2].dma_start(
                    out=out_ext[co * 128:(co + 1) * 128, :], in_=o[:])

    nc.compile()
    return nc


_BUILT = {}


def _get_nc():
    if "nc" not in _BUILT:
        _BUILT["nc"] = build_kernel()
    return _BUILT["nc"]


def make_in_maps(x, Wq, bq, Wk, bk, W6, b6):
    import ml_dtypes
    e4 = ml_dtypes.float8_e4m3
    bfl = ml_dtypes.bfloat16
    x = np.asarray(x, dtype=np.float32).reshape(B, C, HW)
    Wq = np.asarray(Wq, np.float32)
    Wk = np.asarray(Wk, np.float32)
    bq = np.asarray(bq, np.float32)
    bk = np.asarray(bk, np.float32)
    w6t = np.ascontiguousarray(np.asarray(W6, np.float32).T).astype(bfl)
    b6c = np.ascontiguousarray(np.asarray(b6, np.float32).reshape(C, 1))

    # host-folded score factorization
    Wqk = Wk.T @ Wq                       # [c_tilde, c_in]
    bqk = (Wk.T @ bq).reshape(C, 1)
    hbk = (Wq.T @ bk).reshape(C, 1)

    def dr_pack(ws):
        """[c(contraction), M] fp32 -> G x [128, 2*M] fp8 plane-paired."""
        M = ws.shape[1]
        w8 = ws.astype(e4).reshape(G, 2, 128, M)
        return [np.ascontiguousarray(
            np.transpose(w8[g], (1, 0, 2)).reshape(128, 2 * M))
            for g in range(G)]

    wqk8g = dr_pack(Wqk.T * WQK_SCALE)    # lhsT layout [c_in, c_tilde]
    hbk8g = dr_pack(hbk * HBK_SCALE)      # [c_in, 1]
    # fp8 DoubleRow layouts of x for every core, image-rolled so slot 0 is
    # the core's own image: c = g*256 + i*128 + p
    xc = np.transpose(x, (1, 0, 2))                      # [c, img, hw]
    x8_f = xc.astype(e4)
    maps = []
    for b in range(B):
        order = [(b + i) % B for i in range(B)]
        xr = x8_f[:, order, :].reshape(G, 2, 128, B * HW)
        x8g = [np.ascontiguousarray(
            np.transpose(xr[g], (1, 0, 2)).reshape(128, 2 * B * HW))
            for g in range(G)]
        m = {"x": np.ascontiguousarray(x[b]).astype(bfl),
             "w6t": w6t, "bqk": bqk, "b6": b6c}
        for g in range(G):
            m[f"x8g{g}"] = x8g[g]
            m[f"wqk8g{g}"] = wqk8g[g]
            m[f"hbk8g{g}"] = hbk8g[g]
        maps.append(m)
    return maps


def kernel(x, Wq, bq, Wk, bk, W6, b6, _trace=False):
    from concourse import bass_utils
    nc = _get_nc()
    in_maps = make_in_maps(x, Wq, bq, Wk, bk, W6, b6)
    res = bass_utils.run_bass_kernel_spmd(
        nc, in_maps, core_ids=list(range(NCORES)), trace=_trace)
    out = np.stack([np.asarray(res.results[i]["out"]) for i in range(NCORES)])
    out = out.reshape(B, C, H, W).astype(np.float32)
    if _trace:
        return out, res
    return out
